# revision 6
# baseline (speedup 1.0000x reference)
"""Additive attention (nn_AdditiveAttention) distributed Bass kernel for 8 TRN2 cores.

Reference math (per batch b):
    k = key @ Wk                  (NK, H)
    q = query @ Wq                (NQ, H)
    scores[ki, qi] = sum_h wv[h] * tanh(k[ki, h] + q[qi, h])
    masked = where(qi < valid_lens[b], scores, -1e6)
    attn = softmax(masked, axis=qi)
    out = attn @ value            (NK, DV)

Key facts used:
  * Masked q-columns produce attn == 0 exactly (exp(-1e6 - m) underflows to 0 in
    fp32), so columns qi >= valid_len contribute nothing to output or denominator.
    The kernel therefore only processes qi < Ts where Ts = per-slot trip count =
    max valid_len over the cores' slot-s batches, rounded up to the chunk size.
  * softmax without max-subtraction is safe: |scores| <= sum|wv| ~ 10.

Sharding: data-parallel over batch. Each core processes 2 batches ("slots");
slot 0 gets the 8 largest valid_lens, slot 1 the 8 smallest, so the SPMD-static
trip counts (T0, T1) stay near sum(vl)/8 of real work.

Per-q work (the dominant cost), chunked by CH=32 q-columns:
  DVE:     sum[:, j, :] = kT(bf16) + qT[:, q]      (tensor_scalar add)
  ScalarE: tanh in-place over the whole (128, CH*256) chunk (one big ACTIVATE)
  TensorE: per q: scores column = feat_blk^T(bf16) @ wv into PSUM (k-part, q-free)
"""

import numpy as np

import concourse.bass as bass
import concourse.bacc as bacc
import concourse.tile as tile
from concourse import mybir
from concourse.bass_utils import run_bass_kernel_spmd

B = 16
NK = 256
NQ = 256
DK = 256
DV = 256
H = 128
P = 128
NCORES = 8
SLOTS = 2
CH = 32  # q-columns per tanh chunk
MASK_VAL = -1000000.0

F32 = mybir.dt.float32
BF16 = mybir.dt.bfloat16
I32 = mybir.dt.int32
TANH = mybir.ActivationFunctionType.Tanh
EXP = mybir.ActivationFunctionType.Exp
ADD = mybir.AluOpType.add

_CACHE = {}


def _qblocks(t):
    """Split t query-rows into PE-contraction blocks of <=128 rows."""
    blocks = []
    off = 0
    while off < t:
        n = min(P, t - off)
        blocks.append((off, n))
        off += n
    return blocks


def _build(trips):
    nc = bacc.Bacc("TRN2", target_bir_lowering=False, debug=False, num_devices=NCORES)

    key_d = nc.dram_tensor("keyx", [SLOTS, NK, DK], F32, kind="ExternalInput")
    query_d = nc.dram_tensor("queryx", [SLOTS, NQ, DK], F32, kind="ExternalInput")
    value_d = nc.dram_tensor("valuex", [SLOTS, NQ, DV], F32, kind="ExternalInput")
    vlf_d = nc.dram_tensor("vlf", [SLOTS], F32, kind="ExternalInput")
    wk_d = nc.dram_tensor("Wk", [DK, H], F32, kind="ExternalInput")
    wq_d = nc.dram_tensor("Wq", [DK, H], F32, kind="ExternalInput")
    wv_d = nc.dram_tensor("wv", [H, 1], F32, kind="ExternalInput")
    id_d = nc.dram_tensor("ident", [P, P], F32, kind="ExternalInput")
    out_d = nc.dram_tensor("out", [SLOTS, NK, DV], F32, kind="ExternalOutput")

    NKB = NK // P
    DKB = DK // P
    QMAX = max(trips)

    with tile.TileContext(nc) as tc:
        with (
            tc.tile_pool(name="const", bufs=1) as const,
            tc.tile_pool(name="big", bufs=1) as big,
            tc.tile_pool(name="work", bufs=2) as work,
            tc.tile_pool(name="chunk", bufs=3) as chunk_pool,
            tc.tile_pool(name="ps_sc", bufs=2, space="PSUM") as ps_sc,
            tc.tile_pool(name="ps_tmp", bufs=2, space="PSUM") as ps_tmp,
        ):
            # ---- tiles ----
            wkf = const.tile([P, DKB, H], F32)
            wqf = const.tile([P, DKB, H], F32)
            wk_sb = const.tile([P, DKB, H], BF16)
            wq_sb = const.tile([P, DKB, H], BF16)
            wvf = const.tile([P, 1], F32)
            wv_sb = const.tile([P, 1], BF16)
            idf = const.tile([P, P], F32)
            id_sb = const.tile([P, P], BF16)
            vl_sb = const.tile([P, SLOTS], F32)
            iota_sb = const.tile([P, NQ], I32)
            iotaf_sb = const.tile([P, NQ], F32)
            mask_sb = const.tile([P, SLOTS, NQ], F32)

            kT_sb = big.tile([P, SLOTS, NK], BF16)
            qT_sb = big.tile([P, SLOTS, NQ], F32)
            val_sb = big.tile([P, SLOTS, NQ // P, DV], BF16)
            attn_sb = big.tile([P, SLOTS, NKB, QMAX], BF16)
            attnT_sb = big.tile([P, SLOTS, (QMAX + P - 1) // P, NK], BF16)
            rec_sb = big.tile([P, SLOTS, NKB], F32)

            natf = {}
            for s in range(SLOTS):
                for name in ("k", "q"):
                    natf[name, s] = work.tile(
                        [P, NKB, DK], F32, name=f"natf_{name}{s}", tag=f"natf_{name}{s}"
                    )

            # ---- DMAs: critical loads first, split across sync + gpsimd queues ----
            nc.sync.dma_start(out=wvf, in_=wv_d[:, :])
            nc.sync.dma_start(out=idf, in_=id_d[:, :])
            for nb in range(NKB):
                nc.gpsimd.dma_start(
                    out=natf["k", 0][:, nb, :], in_=key_d[0, nb * P : (nb + 1) * P, :]
                )
            for i in range(DKB):
                nc.sync.dma_start(out=wkf[:, i, :], in_=wk_d[i * P : (i + 1) * P, :])
            for nb in range(NKB):
                nc.gpsimd.dma_start(
                    out=natf["q", 0][:, nb, :], in_=query_d[0, nb * P : (nb + 1) * P, :]
                )
            for i in range(DKB):
                nc.sync.dma_start(out=wqf[:, i, :], in_=wq_d[i * P : (i + 1) * P, :])
            for nb in range(NKB):
                nc.gpsimd.dma_start(
                    out=natf["k", 1][:, nb, :], in_=key_d[1, nb * P : (nb + 1) * P, :]
                )
                nc.gpsimd.dma_start(
                    out=natf["q", 1][:, nb, :], in_=query_d[1, nb * P : (nb + 1) * P, :]
                )
            vlf_ap = vlf_d.ap()
            vlf_bcast = bass.AP(
                tensor=vlf_ap.tensor, offset=vlf_ap.offset, ap=[[0, P]] + list(vlf_ap.ap)
            )
            nc.sync.dma_start(out=vl_sb, in_=vlf_bcast)


            # ---- phase A (all fp32 on PE; outputs converted at the final copy) ----
            def phase_a(s):
                for name, wf, dstT in (("k", wkf, kT_sb), ("q", wqf, qT_sb)):
                    nat = natf[name, s]
                    xT = work.tile(
                        [P, DKB, NK], F32, name=f"xT_{name}{s}", tag=f"xT_{name}{s}"
                    )
                    for nb in range(NKB):
                        for db in range(DKB):
                            tp = ps_tmp.tile(
                                [P, P], F32, name=f"tp_{name}{s}{nb}{db}", tag="tp"
                            )
                            nc.tensor.transpose(
                                tp, nat[:, nb, db * P : (db + 1) * P], idf
                            )
                            nc.vector.tensor_copy(xT[:, db, nb * P : (nb + 1) * P], tp)
                    prj = ps_tmp.tile([P, NK], F32, name=f"prj_{name}{s}", tag="prj")
                    for db in range(DKB):
                        nc.tensor.matmul(
                            prj,
                            wf[:, db, :],
                            xT[:, db, :],
                            start=(db == 0),
                            stop=(db == DKB - 1),
                        )
                    if name == "q":
                        # split so the first chunk's adds can start early
                        nc.scalar.copy(dstT[:, s, :CH], prj[:, :CH])
                        nc.scalar.copy(dstT[:, s, CH:], prj[:, CH:])
                    else:
                        nc.scalar.copy(dstT[:, s, :], prj)

            phase_a(0)
            phase_a(1)
            nc.vector.tensor_copy(id_sb, idf)
            nc.vector.tensor_copy(wv_sb, wvf)

            # ---- deferred const prep + value loads (off the critical path) ----
            nc.gpsimd.iota(iota_sb, pattern=[[1, NQ]], base=0, channel_multiplier=0)
            nc.vector.tensor_copy(iotaf_sb, iota_sb)
            for s in range(SLOTS):
                nc.vector.tensor_scalar(
                    out=mask_sb[:, s, :],
                    in0=iotaf_sb,
                    scalar1=vl_sb[:, s : s + 1],
                    scalar2=MASK_VAL,
                    op0=mybir.AluOpType.is_ge,
                    op1=mybir.AluOpType.mult,
                )
            for s in range(SLOTS):
                for qb in range(NQ // P):
                    vf = work.tile([P, DV], F32, name=f"vf{s}{qb}", tag="vf")
                    nc.gpsimd.dma_start(
                        out=vf, in_=value_d[s, qb * P : (qb + 1) * P, :]
                    )
                    nc.vector.tensor_copy(val_sb[:, s, qb, :], vf)

            # ---- phase B+C per slot ----
            for s in range(SLOTS):
                T = trips[s]
                sc_ps = [
                    ps_sc.tile([P, QMAX], F32, name=f"sc{s}{kb}", tag=f"sc{kb}")
                    for kb in range(NKB)
                ]
                if s == 0:
                    sizes = [8, 8, 16]
                else:
                    sizes = []
                done = sum(sizes)
                while done < T:
                    g = min(CH, T - done)
                    sizes.append(g)
                    done += g
                if s == SLOTS - 1 and sizes[-1] == CH:
                    sizes[-1] = CH // 2
                    sizes.append(CH // 2)
                offs = [sum(sizes[:i]) for i in range(len(sizes))]
                for c0, g in zip(offs, sizes):
                    sum_t = chunk_pool.tile([P, CH, NK], BF16, name="sum_t", tag="sum")
                    for j in range(g):
                        nc.vector.tensor_scalar_add(
                            out=sum_t[:, j, :],
                            in0=kT_sb[:, s, :],
                            scalar1=qT_sb[:, s, c0 + j : c0 + j + 1],
                        )
                    nc.scalar.activation(
                        out=sum_t[:, :g, :], in_=sum_t[:, :g, :], func=TANH
                    )
                    for j in range(g):
                        for kb in range(NKB):
                            nc.tensor.matmul(
                                sc_ps[kb][:, c0 + j : c0 + j + 1],
                                sum_t[:, j, kb * P : (kb + 1) * P],
                                wv_sb,
                                start=True,
                                stop=True,
                            )

                # ---- mask + exp + rowsum ----
                den = work.tile([P, NKB], F32, name=f"den{s}", tag="den")
                for kb in range(NKB):
                    nc.vector.tensor_tensor(
                        out=sc_ps[kb][:, :T],
                        in0=sc_ps[kb][:, :T],
                        in1=mask_sb[:, s, :T],
                        op=ADD,
                    )
                    nc.scalar.activation(
                        out=attn_sb[:, s, kb, :T],
                        in_=sc_ps[kb][:, :T],
                        func=EXP,
                        accum_out=den[:, kb : kb + 1],
                    )
                nc.vector.reciprocal(rec_sb[:, s, :], den)

                # ---- attn^T via PE transpose ----
                qblocks = _qblocks(T)
                for kb in range(NKB):
                    for qb, (off, n) in enumerate(qblocks):
                        tp2 = ps_tmp.tile([P, P], BF16, name=f"tp2_{s}{kb}{qb}", tag="tp")
                        nc.tensor.transpose(
                            tp2[:n, :], attn_sb[:, s, kb, off : off + n], id_sb
                        )
                        nc.vector.tensor_copy(
                            attnT_sb[:n, s, qb, kb * P : (kb + 1) * P], tp2[:n, :]
                        )

                # ---- attn @ value + row normalize ----
                for kb in range(NKB):
                    av = ps_tmp.tile([P, DV], F32, name=f"av{s}{kb}", tag="prj")
                    for qb, (off, n) in enumerate(qblocks):
                        nc.tensor.matmul(
                            av,
                            attnT_sb[:n, s, qb, kb * P : (kb + 1) * P],
                            val_sb[:, s, off // P, :][0:n, :],
                            start=(qb == 0),
                            stop=(qb == len(qblocks) - 1),
                        )
                    o_sb = work.tile([P, DV], F32, name=f"o{s}{kb}", tag="o")
                    nc.vector.tensor_scalar(
                        out=o_sb,
                        in0=av,
                        scalar1=rec_sb[:, s, kb : kb + 1],
                        scalar2=None,
                        op0=mybir.AluOpType.mult,
                    )
                    nc.sync.dma_start(out=out_d[s, kb * P : (kb + 1) * P, :], in_=o_sb)

    nc.compile()
    return nc


def kernel(key, query, value, valid_lens, Wk, Wq, wv, _trace=False):
    key = np.ascontiguousarray(np.asarray(key, dtype=np.float32))
    query = np.ascontiguousarray(np.asarray(query, dtype=np.float32))
    value = np.ascontiguousarray(np.asarray(value, dtype=np.float32))
    valid_lens = np.asarray(valid_lens)
    Wk = np.ascontiguousarray(np.asarray(Wk, dtype=np.float32))
    Wq = np.ascontiguousarray(np.asarray(Wq, dtype=np.float32))
    wv = np.ascontiguousarray(np.asarray(wv, dtype=np.float32)).reshape(H, 1)
    ident = np.eye(P, dtype=np.float32)

    vl = np.clip(valid_lens.astype(np.int64), 1, NQ)
    order = np.argsort(-vl, kind="stable")  # descending
    slot0 = order[:NCORES]
    slot1 = order[NCORES:][::-1]
    assign = list(zip(slot0.tolist(), slot1.tolist()))

    def _trip(batches):
        m = int(vl[batches].max())
        return min(NQ, -(-m // CH) * CH)

    trips = (_trip(slot0), _trip(slot1))

    if trips not in _CACHE:
        _CACHE[trips] = _build(trips)
    nc = _CACHE[trips]

    in_maps = []
    for b0, b1 in assign:
        ids = [b0, b1]
        in_maps.append(
            {
                "keyx": key[ids],
                "queryx": query[ids],
                "valuex": value[ids],
                "vlf": valid_lens[ids].astype(np.float32),
                "Wk": Wk,
                "Wq": Wq,
                "wv": wv,
                "ident": ident,
            }
        )

    res = run_bass_kernel_spmd(nc, in_maps, core_ids=list(range(NCORES)), trace=_trace)
    kernel.last_results = res

    out = np.empty((B, NK, DV), dtype=np.float32)
    for c, (b0, b1) in enumerate(assign):
        shard = res.results[c]["out"]
        out[b0] = shard[0]
        out[b1] = shard[1]
    return out


# revision 7
# speedup vs baseline: 1.2036x; 1.2036x over previous
"""Additive attention (nn_AdditiveAttention) distributed Bass kernel for 8 TRN2 cores.

Reference math (per batch b):
    k = key @ Wk                  (NK, H)
    q = query @ Wq                (NQ, H)
    scores[ki, qi] = sum_h wv[h] * tanh(k[ki, h] + q[qi, h])
    masked = where(qi < valid_lens[b], scores, -1e6)
    attn = softmax(masked, axis=qi)
    out = attn @ value            (NK, DV)

Key facts used:
  * Masked q-columns produce attn == 0 exactly (exp(-1e6 - m) underflows to 0 in
    fp32), so columns qi >= valid_len contribute nothing to output or denominator.
    The kernel therefore only processes qi < Ts where Ts = per-slot trip count =
    max valid_len over the cores' slot-s batches, rounded up to the chunk size.
  * softmax without max-subtraction is safe: |scores| <= sum|wv| ~ 10.

Sharding: data-parallel over batch. Each core processes 2 batches ("slots");
slot 0 gets the 8 largest valid_lens, slot 1 the 8 smallest, so the SPMD-static
trip counts (T0, T1) stay near sum(vl)/8 of real work.

Per-q work (the dominant cost), chunked by CH=32 q-columns:
  DVE:     sum[:, j, :] = kT(bf16) + qT[:, q]      (tensor_scalar add)
  ScalarE: tanh in-place over the whole (128, CH*256) chunk (one big ACTIVATE)
  TensorE: per q: scores column = feat_blk^T(bf16) @ wv into PSUM (k-part, q-free)
"""

import numpy as np

import concourse.bass as bass
import concourse.bacc as bacc
import concourse.tile as tile
from concourse import mybir
from concourse.bass_utils import run_bass_kernel_spmd

B = 16
NK = 256
NQ = 256
DK = 256
DV = 256
H = 128
P = 128
NCORES = 8
SLOTS = 2
CH = 32  # q-columns per tanh chunk
MASK_VAL = -1000000.0

F32 = mybir.dt.float32
BF16 = mybir.dt.bfloat16
I32 = mybir.dt.int32
TANH = mybir.ActivationFunctionType.Tanh
EXP = mybir.ActivationFunctionType.Exp
ADD = mybir.AluOpType.add

_CACHE = {}


def _qblocks(t):
    """Split t query-rows into PE-contraction blocks of <=128 rows."""
    blocks = []
    off = 0
    while off < t:
        n = min(P, t - off)
        blocks.append((off, n))
        off += n
    return blocks


def _build(trips):
    nc = bacc.Bacc("TRN2", target_bir_lowering=False, debug=False, num_devices=NCORES)

    key_d = nc.dram_tensor("keyx", [SLOTS, NK, DK], F32, kind="ExternalInput")
    query_d = nc.dram_tensor("queryx", [SLOTS, NQ, DK], F32, kind="ExternalInput")
    value_d = nc.dram_tensor("valuex", [SLOTS, NQ, DV], F32, kind="ExternalInput")
    vlf_d = nc.dram_tensor("vlf", [SLOTS], F32, kind="ExternalInput")
    wk_d = nc.dram_tensor("Wk", [DK, H], F32, kind="ExternalInput")
    wq_d = nc.dram_tensor("Wq", [DK, H], F32, kind="ExternalInput")
    wv_d = nc.dram_tensor("wv", [H, 1], F32, kind="ExternalInput")
    id_d = nc.dram_tensor("ident", [P, P], F32, kind="ExternalInput")
    out_d = nc.dram_tensor("out", [SLOTS, NK, DV], F32, kind="ExternalOutput")

    NKB = NK // P
    DKB = DK // P
    QMAX = max(trips)

    with tile.TileContext(nc) as tc:
        with (
            tc.tile_pool(name="const", bufs=1) as const,
            tc.tile_pool(name="big", bufs=1) as big,
            tc.tile_pool(name="work", bufs=2) as work,
            tc.tile_pool(name="chunk", bufs=3) as chunk_pool,
            tc.tile_pool(name="ps_sc", bufs=2, space="PSUM") as ps_sc,
            tc.tile_pool(name="ps_tmp", bufs=2, space="PSUM") as ps_tmp,
        ):
            # ---- tiles ----
            wkf = const.tile([P, DKB, H], F32)
            wqf = const.tile([P, DKB, H], F32)
            wk_sb = const.tile([P, DKB, H], BF16)
            wq_sb = const.tile([P, DKB, H], BF16)
            wvf = const.tile([P, 1], F32)
            wv_sb = const.tile([P, 1], BF16)
            idf = const.tile([P, P], F32)
            id_sb = const.tile([P, P], BF16)
            vl_sb = const.tile([P, SLOTS], F32)
            iota_sb = const.tile([P, NQ], I32)
            iotaf_sb = const.tile([P, NQ], F32)
            mask_sb = const.tile([P, SLOTS, NQ], F32)

            kT_sb = big.tile([P, SLOTS, NK], BF16)
            qT_sb = big.tile([P, SLOTS, NQ], F32)
            val_sb = big.tile([P, SLOTS, NQ // P, DV], BF16)
            attn_sb = big.tile([P, SLOTS, NKB, QMAX], BF16)
            attnT_sb = big.tile([P, SLOTS, (QMAX + P - 1) // P, NK], BF16)
            rec_sb = big.tile([P, SLOTS, NKB], F32)

            natf = {}
            for s in range(SLOTS):
                for name in ("k", "q"):
                    natf[name, s] = work.tile(
                        [P, NKB, DK], F32, name=f"natf_{name}{s}", tag=f"natf_{name}{s}"
                    )

            # ---- DMAs: critical loads first, split across sync + gpsimd queues ----
            nc.sync.dma_start(out=wvf, in_=wv_d[:, :])
            nc.sync.dma_start(out=idf, in_=id_d[:, :])
            for nb in range(NKB):
                nc.gpsimd.dma_start(
                    out=natf["k", 0][:, nb, :], in_=key_d[0, nb * P : (nb + 1) * P, :]
                )
            for i in range(DKB):
                nc.sync.dma_start(out=wkf[:, i, :], in_=wk_d[i * P : (i + 1) * P, :])
            for nb in range(NKB):
                nc.gpsimd.dma_start(
                    out=natf["q", 0][:, nb, :], in_=query_d[0, nb * P : (nb + 1) * P, :]
                )
            for i in range(DKB):
                nc.sync.dma_start(out=wqf[:, i, :], in_=wq_d[i * P : (i + 1) * P, :])
            for nb in range(NKB):
                nc.gpsimd.dma_start(
                    out=natf["k", 1][:, nb, :], in_=key_d[1, nb * P : (nb + 1) * P, :]
                )
                nc.gpsimd.dma_start(
                    out=natf["q", 1][:, nb, :], in_=query_d[1, nb * P : (nb + 1) * P, :]
                )
            vlf_ap = vlf_d.ap()
            vlf_bcast = bass.AP(
                tensor=vlf_ap.tensor, offset=vlf_ap.offset, ap=[[0, P]] + list(vlf_ap.ap)
            )
            nc.sync.dma_start(out=vl_sb, in_=vlf_bcast)


            # ---- early casts (critical path for slot-0 projections) ----
            nc.vector.tensor_copy(id_sb, idf)
            nc.vector.tensor_copy(wv_sb, wvf)

            # ---- phase A: per slot, kT/qT = (x @ W)^T ----
            def phase_a(s):
                for name, w_sb, wf, dstT in (
                    ("k", wk_sb, wkf, kT_sb),
                    ("q", wq_sb, wqf, qT_sb),
                ):
                    nat = work.tile(
                        [P, NKB, DK], BF16, name=f"nat_{name}{s}", tag=f"nat_{name}{s}"
                    )
                    nc.vector.tensor_copy(nat[:, :, :], natf[name, s][:, :, :])
                    if s == 0:
                        nc.vector.tensor_copy(w_sb[:, :, :], wf[:, :, :])
                    xT = work.tile(
                        [P, DKB, NK], BF16, name=f"xT_{name}{s}", tag=f"xT_{name}{s}"
                    )
                    for nb in range(NKB):
                        for db in range(DKB):
                            tp = ps_tmp.tile(
                                [P, P], BF16, name=f"tp_{name}{s}{nb}{db}", tag="tp"
                            )
                            nc.tensor.transpose(
                                tp, nat[:, nb, db * P : (db + 1) * P], id_sb
                            )
                            nc.vector.tensor_copy(xT[:, db, nb * P : (nb + 1) * P], tp)
                    prj = ps_tmp.tile([P, NK], F32, name=f"prj_{name}{s}", tag="prj")
                    for db in range(DKB):
                        nc.tensor.matmul(
                            prj,
                            w_sb[:, db, :],
                            xT[:, db, :],
                            start=(db == 0),
                            stop=(db == DKB - 1),
                        )
                    if name == "q":
                        nc.scalar.copy(dstT[:, s, :CH], prj[:, :CH])
                        nc.scalar.copy(dstT[:, s, CH:], prj[:, CH:])
                    else:
                        nc.scalar.copy(dstT[:, s, :], prj)

            phase_a(0)
            phase_a(1)

            # ---- deferred const prep + value loads (off the critical path) ----
            nc.gpsimd.iota(iota_sb, pattern=[[1, NQ]], base=0, channel_multiplier=0)
            nc.vector.tensor_copy(iotaf_sb, iota_sb)
            for s in range(SLOTS):
                nc.vector.tensor_scalar(
                    out=mask_sb[:, s, :],
                    in0=iotaf_sb,
                    scalar1=vl_sb[:, s : s + 1],
                    scalar2=MASK_VAL,
                    op0=mybir.AluOpType.is_ge,
                    op1=mybir.AluOpType.mult,
                )
            for s in range(SLOTS):
                for qb in range(NQ // P):
                    vf = work.tile([P, DV], F32, name=f"vf{s}{qb}", tag="vf")
                    nc.gpsimd.dma_start(
                        out=vf, in_=value_d[s, qb * P : (qb + 1) * P, :]
                    )
                    nc.vector.tensor_copy(val_sb[:, s, qb, :], vf)

            # ---- phase B+C per slot ----
            for s in range(SLOTS):
                T = trips[s]
                sc_ps = [
                    ps_sc.tile([P, QMAX], F32, name=f"sc{s}{kb}", tag=f"sc{kb}")
                    for kb in range(NKB)
                ]
                if s == 0:
                    sizes = [8, 8, 16]
                else:
                    sizes = []
                done = sum(sizes)
                while done < T:
                    g = min(CH, T - done)
                    sizes.append(g)
                    done += g
                if s == SLOTS - 1 and sizes[-1] == CH:
                    sizes[-1] = CH // 2
                    sizes.append(CH // 2)
                offs = [sum(sizes[:i]) for i in range(len(sizes))]
                for c0, g in zip(offs, sizes):
                    sum_t = chunk_pool.tile([P, CH, NK], BF16, name="sum_t", tag="sum")
                    for j in range(g):
                        nc.vector.tensor_scalar_add(
                            out=sum_t[:, j, :],
                            in0=kT_sb[:, s, :],
                            scalar1=qT_sb[:, s, c0 + j : c0 + j + 1],
                        )
                    nc.scalar.activation(
                        out=sum_t[:, :g, :], in_=sum_t[:, :g, :], func=TANH
                    )
                    for j in range(g):
                        for kb in range(NKB):
                            nc.tensor.matmul(
                                sc_ps[kb][:, c0 + j : c0 + j + 1],
                                sum_t[:, j, kb * P : (kb + 1) * P],
                                wv_sb,
                                start=True,
                                stop=True,
                            )

                # ---- mask + exp + rowsum ----
                den = work.tile([P, NKB], F32, name=f"den{s}", tag="den")
                for kb in range(NKB):
                    nc.vector.tensor_tensor(
                        out=sc_ps[kb][:, :T],
                        in0=sc_ps[kb][:, :T],
                        in1=mask_sb[:, s, :T],
                        op=ADD,
                    )
                    nc.scalar.activation(
                        out=attn_sb[:, s, kb, :T],
                        in_=sc_ps[kb][:, :T],
                        func=EXP,
                        accum_out=den[:, kb : kb + 1],
                    )
                nc.vector.reciprocal(rec_sb[:, s, :], den)

                # ---- attn^T via PE transpose ----
                qblocks = _qblocks(T)
                for kb in range(NKB):
                    for qb, (off, n) in enumerate(qblocks):
                        tp2 = ps_tmp.tile([P, P], BF16, name=f"tp2_{s}{kb}{qb}", tag="tp")
                        nc.tensor.transpose(
                            tp2[:n, :], attn_sb[:, s, kb, off : off + n], id_sb
                        )
                        nc.vector.tensor_copy(
                            attnT_sb[:n, s, qb, kb * P : (kb + 1) * P], tp2[:n, :]
                        )

                # ---- attn @ value + row normalize ----
                for kb in range(NKB):
                    av = ps_tmp.tile([P, DV], F32, name=f"av{s}{kb}", tag="prj")
                    for qb, (off, n) in enumerate(qblocks):
                        nc.tensor.matmul(
                            av,
                            attnT_sb[:n, s, qb, kb * P : (kb + 1) * P],
                            val_sb[:, s, off // P, :][0:n, :],
                            start=(qb == 0),
                            stop=(qb == len(qblocks) - 1),
                        )
                    o_sb = work.tile([P, DV], F32, name=f"o{s}{kb}", tag="o")
                    nc.vector.tensor_scalar(
                        out=o_sb,
                        in0=av,
                        scalar1=rec_sb[:, s, kb : kb + 1],
                        scalar2=None,
                        op0=mybir.AluOpType.mult,
                    )
                    nc.sync.dma_start(out=out_d[s, kb * P : (kb + 1) * P, :], in_=o_sb)

    nc.compile()
    return nc


def kernel(key, query, value, valid_lens, Wk, Wq, wv, _trace=False):
    key = np.ascontiguousarray(np.asarray(key, dtype=np.float32))
    query = np.ascontiguousarray(np.asarray(query, dtype=np.float32))
    value = np.ascontiguousarray(np.asarray(value, dtype=np.float32))
    valid_lens = np.asarray(valid_lens)
    Wk = np.ascontiguousarray(np.asarray(Wk, dtype=np.float32))
    Wq = np.ascontiguousarray(np.asarray(Wq, dtype=np.float32))
    wv = np.ascontiguousarray(np.asarray(wv, dtype=np.float32)).reshape(H, 1)
    ident = np.eye(P, dtype=np.float32)

    vl = np.clip(valid_lens.astype(np.int64), 1, NQ)
    order = np.argsort(-vl, kind="stable")  # descending
    slot0 = order[:NCORES]
    slot1 = order[NCORES:][::-1]
    assign = list(zip(slot0.tolist(), slot1.tolist()))

    def _trip(batches):
        m = int(vl[batches].max())
        return min(NQ, -(-m // CH) * CH)

    trips = (_trip(slot0), _trip(slot1))

    if trips not in _CACHE:
        _CACHE[trips] = _build(trips)
    nc = _CACHE[trips]

    in_maps = []
    for b0, b1 in assign:
        ids = [b0, b1]
        in_maps.append(
            {
                "keyx": key[ids],
                "queryx": query[ids],
                "valuex": value[ids],
                "vlf": valid_lens[ids].astype(np.float32),
                "Wk": Wk,
                "Wq": Wq,
                "wv": wv,
                "ident": ident,
            }
        )

    res = run_bass_kernel_spmd(nc, in_maps, core_ids=list(range(NCORES)), trace=_trace)
    kernel.last_results = res

    out = np.empty((B, NK, DV), dtype=np.float32)
    for c, (b0, b1) in enumerate(assign):
        shard = res.results[c]["out"]
        out[b0] = shard[0]
        out[b1] = shard[1]
    return out
